# revision 67
# baseline (speedup 1.0000x reference)
"""Trainium2 Bass kernel for nn_MultiDomainPLEFENDModel (soft-MoE multi-domain FEND).

Strategy (8 NeuronCores, SPMD):
  - Heavy compute = 60 stacked CNN extractors x 2 modalities (~1.5 TFLOP).
    Conv expressed as accumulated PE matmuls over (d-chunk-pair, tap) in
    fp8e4 DoubleRow mode (2 MACs/cell/cycle), fp32 PSUM accumulation,
    fused max-pool on DVE. Inputs scaled x16, weights x256 into the fp8
    range; the 4096x feature scale is folded into cbias (x4096) and
    cw1 (/4096) so everything downstream of the combine is exact.
  - Conv streams use full 200-token sample windows so the (2-sample, l)
    dims merge into one contiguous 400-col moving operand; the max-pool
    reduce only reads the valid [0, lo) columns.
  - Expert-parallel sharding: core c owns domain c's 6 experts for both
    modalities plus one shared-expert row-tile per modality (12 shared
    tiles spread over the 8 cores). Shared features are AllGather'ed.
  - Each core runs attention-pooling, gating, soft-MoE combine and the
    expert MLP for its own domain only; the host selects each sample's
    domain logit and applies the sigmoid.

Token layout uses a padded flat index r = b*200 + l (BLP = 6400 = 50*128).
"""

import numpy as np
import ml_dtypes

import concourse.bass as bass
import concourse.tile as tile
from concourse import bacc, mybir
from concourse import bass_utils

BF16 = ml_dtypes.bfloat16
E4M3 = ml_dtypes.float8_e4m3
F32 = mybir.dt.float32
BF = mybir.dt.bfloat16
FP8 = mybir.dt.float8e4
DR = mybir.MatmulPerfMode.DoubleRow
ALU = mybir.AluOpType
ACTF = mybir.ActivationFunctionType

B, L, D = 32, 197, 768
LP = 200
BLP = B * LP          # 6400
BLP2 = BLP + 16       # conv windows for the last sample read past BLP
HB = B // 2           # 16 samples per shared-slot batch half
HW2 = HB * LP + 16    # 3216 columns per half-batch buffer
WS = 100              # scores/pool window size
WN = BLP // WS        # 64 windows
TC = BLP // 128       # 50 pool token-chunks of 128
DC = D // 128         # 6
QC = DC // 2          # 3 d-chunk pairs (DoubleRow contraction pairs)
KS = (1, 2, 3, 5, 10)
FK = 64
GATE_E = 18
NCORES = 8
T = 9                 # 6 full domain slots + 3 half-batch shared slots
NHS = 3               # half-slots per core
MODN = {0: "text", 1: "image"}
NBB = B // 2          # 16 conv column blocks (2 samples each)
HBB = NBB // 2        # 8 blocks per shared half-slot
SX = 16.0             # fp8 input scale
SW = 256.0            # fp8 weight scale
SXW = SX * SW         # product scale folded into cbias/cw1


def slot_experts(c, s):
    """(modality, (e0, e1), batch_half) handled by core c's conv slot s.

    Slots 0-2: text domain-c expert pairs (full batch).
    Slots 3-5: image domain-c expert pairs (full batch).
    Slots 6-8: shared-expert half-slots. The 12 shared tiles (6 text +
    6 image) x 2 batch halves = 24 units, 3 per core: unit g = 3c+(s-6)
    -> tile t = g//2 (t<6 text, else image), batch half h = g%2.
    """
    if s < 3:
        return 0, (6 * c + 2 * s, 6 * c + 2 * s + 1), None
    if s < 6:
        return 1, (6 * c + 2 * (s - 3), 6 * c + 2 * (s - 3) + 1), None
    g = 3 * c + (s - 6)
    t, h = g // 2, g % 2
    mod, tm = (0, t) if t < 6 else (1, t - 6)
    return mod, (48 + 2 * tm, 49 + 2 * tm), h


# ---------------------------------------------------------------------------
# Bass module
# ---------------------------------------------------------------------------

def build_nc(slots=None, ks=None, nbb=NBB, reps=1, no_cc=False):
    """Builds the SPMD module. slots/ks/nbb reducible for simulator tests."""
    slots = list(range(T)) if slots is None else slots
    ks = KS if ks is None else ks

    nc = bacc.Bacc(
        "TRN2",
        target_bir_lowering=False,
        debug=False,
        enable_asserts=False,
        num_devices=NCORES,
    )

    di = {}

    def inp(name, shape, dt):
        di[name] = nc.dram_tensor(name, list(shape), dt, kind="ExternalInput")

    for k in KS:
        inp(f"w_k{k}", (T, 128, k, QC, 2, 128), FP8)
    inp("cbias", (128, T, 5), F32)
    inp("xt_t", (DC, 128, BLP2), FP8)
    inp("xt_i", (DC, 128, BLP2), FP8)
    inp("xt_sh", (NHS, DC, 128, HW2), FP8)
    inp("xn_t", (TC, 128, D), FP8)
    inp("xn_i", (TC, 128, D), FP8)
    inp("b_ind", (128, TC, B), BF)
    inp("rep4", (B, 128), F32)
    inp("mask2", (2, B, LP), F32)
    inp("aw", (128, QC, 2, 16), FP8)   # last dim: mod 0/1 + pad to 16 for
                                       # the DoubleRow weight-AP step%16 rule
    inp("dom_embT", (128, DC, B), BF)
    inp("gw1", (2, 12, 128, DC, 128), BF)
    inp("gb1", (2, B, D), F32)
    inp("gw2", (2, 128, DC, GATE_E), F32)
    inp("gb2", (2, B, GATE_E), F32)
    inp("cw1", (2, 64, 5, 3, 128), F32)
    inp("cb1", (2, 128, 3), F32)
    inp("cw2", (2, 128, 3), F32)
    inp("cb2", (2, 1, 1), F32)
    inp("ident128", (128, 128), F32)

    out_dram = nc.dram_tensor("logits", [2, 1, B], F32, kind="ExternalOutput")

    ag_in = nc.dram_tensor("agin", [128, NHS, 5, HB], F32, kind="Internal")
    ag_out = nc.dram_tensor(
        "agout", [NCORES, 128, NHS, 5, HB], F32, kind="Internal",
        addr_space="Shared")
    # dram bounce to re-chunk softmax probs [B,LP] -> [TC,128] (the direct
    # SBUF->SBUF reshape is unlowerable: 200 does not factor against 128)
    s2d = nc.dram_tensor("s2d", [2, BLP], F32, kind="Internal")

    with tile.TileContext(nc) as tc:
        _program(nc, tc, di, out_dram, ag_in, ag_out, s2d, slots, ks, nbb, reps, no_cc)

    nc.compile()
    return nc


def _program(nc, tc, di, out_dram, ag_in, ag_out, s2d, slots, ks, nbb, reps=1, no_cc=False):
    counter = [0]

    def nm(base):
        counter[0] += 1
        return f"{base}{counter[0]}"

    import contextlib
    with contextlib.ExitStack() as ctx:
        ep = ctx.enter_context
        xt_pool = ep(tc.tile_pool(name="xt", bufs=1))
        wk_pool = ep(tc.tile_pool(name="wk", bufs=2))
        xn_pool = ep(tc.tile_pool(name="xn", bufs=8))
        feat_pool = ep(tc.tile_pool(name="feat", bufs=T))
        sh_pool = ep(tc.tile_pool(name="sh", bufs=2))
        small = ep(tc.tile_pool(name="small", bufs=2))
        small1 = ep(tc.tile_pool(name="small1", bufs=1))
        const_pool = ep(tc.tile_pool(name="const", bufs=1))
        gw1_pool = ep(tc.tile_pool(name="gw1p", bufs=2))
        comb_pool = ep(tc.tile_pool(name="comb", bufs=2))
        combt_pool = ep(tc.tile_pool(name="combt", bufs=6))
        psum_conv = ep(tc.tile_pool(name="pconv", bufs=6, space="PSUM"))
        psum_misc = ep(tc.tile_pool(name="pmisc", bufs=2, space="PSUM"))

        # ---- resident constants ----
        def cget(name, shape, dt, src=None):
            t = const_pool.tile(shape, dt, tag=name)
            if src is not None:
                nc.sync.dma_start(t[:], src)
            return t

        # tiny consts the first conv needs — DMA'd up front
        cbias = cget("cbias", [128, T, 5], F32, di["cbias"][:])
        aw = cget("aw", [128, QC, 2, 16], FP8, di["aw"][:])
        ident = cget("ident", [128, 128], F32, di["ident128"][:])
        # heavier consts, first consumed by gating/combine — allocate now,
        # DMA after the first conv slot is underway (see deferred_consts)
        b_ind = cget("bind", [128, TC, B], BF)
        rep4 = cget("rep4", [B, 128], F32)
        dom_embT = cget("domT", [128, DC, B], BF)
        mask2 = cget("mask2", [B, 2, LP], F32)
        gb1 = cget("gb1", [B, 2, D], F32)
        gw2 = cget("gw2", [128, 2, DC, GATE_E], F32)
        gb2 = cget("gb2", [B, 2, GATE_E], F32)
        cw1 = cget("cw1", [64, 2, 5, 3, 128], F32)
        cb1 = cget("cb1", [128, 2, 3], F32)
        cw2 = cget("cw2", [128, 2, 3], F32)
        cb2 = cget("cb2", [1, 2, 1], F32)

        def deferred_consts():
            nc.sync.dma_start(b_ind[:], di["b_ind"][:])
            nc.sync.dma_start(rep4[:], di["rep4"][:])
            nc.sync.dma_start(dom_embT[:], di["dom_embT"][:])
            for m in (0, 1):
                nc.sync.dma_start(mask2[:, m, :], di["mask2"][m])
                nc.sync.dma_start(gb1[:, m, :], di["gb1"][m])
                nc.sync.dma_start(gw2[:, m, :, :], di["gw2"][m])
                nc.sync.dma_start(gb2[:, m, :], di["gb2"][m])
                nc.sync.dma_start(cw1[:, m, :, :, :], di["cw1"][m])
                nc.sync.dma_start(cb1[:, m, :], di["cb1"][m])
                nc.sync.dma_start(cw2[:, m, :], di["cw2"][m])
                nc.sync.dma_start(cb2[0:1, m, :], di["cb2"][m])
        feat = [feat_pool.tile([128, 5, B], F32, tag="feat", name=nm("feat"))
                for _ in range(T)]
        for s in range(T):
            # half-slots only write cols 0:HB — zero the rest up front
            if s >= 6 or s not in slots or len(ks) < len(KS):
                nc.vector.memset(feat[s][:], 0.0)
        sh_sb = {m: sh_pool.tile([128, 6, 5, B], F32, tag="sh", name=nm("sh"))
                 for m in (0, 1)}
        gate_sb = {}

        def conv_slot(s, xap, nbb_s):
            # fp8 DoubleRow: contract (128 partitions x 2 paired d-chunks)
            # per MM; moving operand = two adjacent 200-token sample windows
            # (contiguous 400 cols, the tail window reads zero-pad). Groups
            # of 3 psum banks, weight-outer within a group so LDWEIGHTS
            # (256 cols) amortizes over 3 streams. xap(q, r0) yields the
            # moving AP (full-batch xt or a half-batch shared buffer).
            # k descending: most PE-work-per-DMA-byte first, so the lead
            # slot hides the input DMA ramp.
            for ki, k in sorted(enumerate(KS), key=lambda t: -t[1]):
                if k not in ks:
                    continue
                lo = L - k + 1
                wk = wk_pool.tile([128, k, QC, 2, 128], FP8, tag="wk")
                nc.sync.dma_start(wk[:], di[f"w_k{k}"][s])
                nw = QC * k
                for g0 in range(0, nbb_s, 3):
                    gbb = list(range(g0, min(g0 + 3, nbb_s)))
                    pts = [psum_conv.tile([128, 2, LP], F32, tag="conv",
                                          name=nm("pt")) for _ in gbb]
                    n = 0
                    for j in range(k):
                        for q in range(QC):
                            for bi, bb in enumerate(gbb):
                                nc.tensor.matmul(
                                    pts[bi][:],
                                    wk[:, j, q, :, :],
                                    xap(q, 2 * bb * LP + j),
                                    start=(n == 0), stop=(n == nw - 1),
                                    perf_mode=DR)
                            n += 1
                    for bi, bb in enumerate(gbb):
                        nc.vector.reduce_max(
                            feat[s][:, ki, 2 * bb:2 * bb + 2],
                            pts[bi][:, :, 0:lo],
                            axis=mybir.AxisListType.X)
            for ki in range(len(KS)):
                if KS[ki] not in ks:
                    continue
                nc.vector.tensor_scalar_add(
                    feat[s][:, ki, :], feat[s][:, ki, :],
                    cbias[:, s, ki:ki + 1])

        def sc_stage1(mod, xt):
            """Scores (fp8 DoubleRow) + masked softmax + P build."""
            # 16 slices of 2 samples each (one psum group per slice tile;
            # start=True zeroes a whole 2KB region, so slices must not
            # share a live bank)
            s2 = small.tile([B, LP], F32, tag="s2", name=nm("s2"))
            for sl in range(16):
                spt = psum_misc.tile([1, 2 * LP], F32, tag="misc",
                                     name=nm("spt"))
                for q in range(QC):
                    nc.tensor.matmul(
                        spt[:], aw[:, q, :, mod:mod + 1],
                        xt[:, 2 * q:2 * q + 2, sl * 2 * LP:(sl + 1) * 2 * LP],
                        start=(q == 0), stop=(q == QC - 1),
                        perf_mode=DR)
                scp = small.tile([1, 2 * LP], F32, tag="scp", name=nm("scp"))
                nc.scalar.copy(scp[:], spt[:])
                nc.sync.dma_start(s2[2 * sl:2 * sl + 2, :], scp[:])
            # descale (fp8 scales) + masked softmax over l
            nc.vector.tensor_scalar_mul(s2[:], s2[:], 1.0 / SXW)
            nc.vector.scalar_tensor_tensor(
                out=s2[:], in0=s2[:], scalar=1e9, in1=mask2[:, mod, :],
                op0=ALU.add, op1=ALU.mult)
            nc.vector.tensor_scalar_sub(s2[:], s2[:], 1e9)
            mx = small.tile([B, 1], F32, tag="mx")
            nc.vector.reduce_max(mx[:], s2[:], axis=mybir.AxisListType.X)
            nc.vector.tensor_scalar_sub(s2[:], s2[:], mx[:, 0:1])
            sm = small.tile([B, 1], F32, tag="sm")
            nc.scalar.activation(s2[:], s2[:], ACTF.Exp, accum_out=sm[:])
            rd = small.tile([B, 1], F32, tag="rd")
            nc.vector.reciprocal(rd[:], sm[:])
            nc.vector.tensor_scalar_mul(s2[:], s2[:], rd[:, 0:1])
            return {"s2": s2}

        def sc_stage2(mod, st):
            """P build + attention pool + gate-MLP input assembly. The
            leading transpose waits on stage1's DVE softmax chain, so a
            conv slot should sit between the stages in program order."""
            # p -> pr [128, TC]: bounce through dram to re-chunk [B,LP]
            # into 128-token chunks (K=128 pool matmuls, 50 chunks vs 64)
            nc.sync.dma_start(s2d[mod], st["s2"][:])
            pT = small.tile([TC, 128], F32, tag="pT", name=nm("pT"))
            nc.sync.dma_start(pT[:], s2d[mod].rearrange("(a c) -> a c", c=128))
            tp2 = psum_misc.tile([128, TC], F32, tag="misc", name=nm("tp2"))
            nc.tensor.transpose(tp2[:], pT[:], ident[0:TC, 0:TC])
            pr = small.tile([128, TC], F32, tag="pr", name=nm("pr"))
            nc.scalar.copy(pr[:], tp2[:])
            # P = b_ind * pr
            P = small1.tile([128, TC, B], BF, tag="P", name=nm("P"))
            for ch in range(TC):
                nc.vector.tensor_scalar_mul(
                    P[:, ch, :], b_ind[:, ch, :], pr[:, ch:ch + 1])
            gin = small1.tile([128, 12, B], BF, tag="ginT", name=nm("gin"))
            nc.sync.dma_start(gin[:, 6:12, :], dom_embT[:])
            pba = psum_misc.tile([B, 512], F32, tag="misc", name=nm("pba"))
            pbb = psum_misc.tile([B, D - 512], F32, tag="misc", name=nm("pbb"))
            for ch in range(TC):
                # fp8 x16 — compensated by gw1's pooled rows /16 on host
                xn = xn_pool.tile([128, D], FP8, tag="xn")
                nc.sync.dma_start(xn[:], di[f"xn_{MODN[mod][0]}"][ch])
                nc.tensor.matmul(pba[:], P[:, ch, :], xn[:, 0:512],
                                 start=(ch == 0), stop=(ch == TC - 1))
                nc.tensor.matmul(pbb[:], P[:, ch, :], xn[:, 512:D],
                                 start=(ch == 0), stop=(ch == TC - 1))
            pb_sb = small1.tile([B, D], F32, tag="pbsb", name=nm("pbsb"))
            nc.scalar.copy(pb_sb[:, 0:512], pba[:])
            nc.scalar.copy(pb_sb[:, 512:D], pbb[:])
            for dcc in range(DC):
                tpp = psum_misc.tile([128, B], F32, tag="misc", name=nm("tpp"))
                nc.tensor.transpose(
                    tpp[:], pb_sb[:, dcc * 128:(dcc + 1) * 128],
                    ident[0:B, 0:B])
                nc.scalar.copy(gin[:, dcc, :], tpp[:])
            st["gin"] = gin

        def sc_stage3(mod, st):
            """Gate MLP + gate softmax."""
            gin = st["gin"]
            hba = psum_misc.tile([B, 512], F32, tag="misc", name=nm("hba"))
            hbb = psum_misc.tile([B, D - 512], F32, tag="misc", name=nm("hbb"))
            for ic in range(12):
                g1 = gw1_pool.tile([128, D], BF, tag="gw1c")
                nc.sync.dma_start(g1[:], di["gw1"][mod, ic])
                nc.tensor.matmul(hba[:], gin[:, ic, :], g1[:, 0:512],
                                 start=(ic == 0), stop=(ic == 11))
                nc.tensor.matmul(hbb[:], gin[:, ic, :], g1[:, 512:D],
                                 start=(ic == 0), stop=(ic == 11))
            h_sb = small1.tile([B, D], F32, tag="hsb", name=nm("hsb"))
            nc.vector.tensor_tensor(
                out=h_sb[:, 0:512], in0=hba[:], in1=gb1[:, mod, 0:512],
                op=ALU.add)
            nc.vector.tensor_tensor(
                out=h_sb[:, 512:D], in0=hbb[:], in1=gb1[:, mod, 512:D],
                op=ALU.add)
            hsg = small1.tile([B, D], F32, tag="hsg", name=nm("hsg"))
            nc.scalar.activation(hsg[:], h_sb[:], ACTF.Sigmoid)
            nc.vector.tensor_tensor(
                out=h_sb[:], in0=h_sb[:], in1=hsg[:], op=ALU.mult)
            hT = small1.tile([128, DC, B], F32, tag="hT", name=nm("hT"))
            for oc in range(DC):
                tph = psum_misc.tile([128, B], F32, tag="misc", name=nm("tph"))
                nc.tensor.transpose(
                    tph[:], h_sb[:, oc * 128:(oc + 1) * 128], ident[0:B, 0:B])
                nc.scalar.copy(hT[:, oc, :], tph[:])
            gl_ps = psum_misc.tile([B, GATE_E], F32, tag="misc",
                                   name=nm("glps"))
            for oc in range(DC):
                nc.tensor.matmul(
                    gl_ps[:], hT[:, oc, :], gw2[:, mod, oc, :],
                    start=(oc == 0), stop=(oc == DC - 1))
            gate = small.tile([B, GATE_E], F32, tag="gate", name=nm("gate"))
            nc.vector.tensor_tensor(
                out=gate[:], in0=gl_ps[:], in1=gb2[:, mod, :], op=ALU.add)
            gmx = small.tile([B, 1], F32, tag="gmx")
            nc.vector.reduce_max(gmx[:], gate[:], axis=mybir.AxisListType.X)
            nc.vector.tensor_scalar_sub(gate[:], gate[:], gmx[:, 0:1])
            gsm = small.tile([B, 1], F32, tag="gsm")
            nc.scalar.activation(gate[:], gate[:], ACTF.Exp, accum_out=gsm[:])
            grd = small.tile([B, 1], F32, tag="grd")
            nc.vector.reciprocal(grd[:], gsm[:])
            nc.vector.tensor_scalar_mul(gate[:], gate[:], grd[:, 0:1])
            return gate

        def combine_mlp(mod, gate):
            dslots = [0, 1, 2] if mod == 0 else [3, 4, 5]
            # gate replicated 4x along partitions: p = ki*32+b holds gate[b]
            g4_ps = psum_misc.tile([128, GATE_E], F32, tag="misc",
                                   name=nm("g4ps"))
            nc.tensor.matmul(g4_ps[:], rep4[:], gate[:], start=True, stop=True)
            gate4 = comb_pool.tile([128, GATE_E], F32, tag="gate4",
                                   name=nm("gate4"))
            nc.scalar.copy(gate4[:], g4_ps[:])
            # combine accumulators in (ki, b)-partition space:
            # comb4[ki*32+b, f] for ki 0-3, combB[b, f] for ki=4
            comb4 = comb_pool.tile([128, 64], F32, tag="comb4",
                                   name=nm("comb4"))
            combB = comb_pool.tile([B, 64], F32, tag="combB", name=nm("combB"))
            nc.vector.memset(comb4[:], 0.0)
            nc.vector.memset(combB[:], 0.0)
            # 9 source row-tiles: 3 domain (local feat) + 6 shared (AG)
            for si in range(9):
                if si < 3:
                    src_ap = feat[dslots[si]][:]                # [128,5,B]
                    e_base = 2 * si
                else:
                    src_ap = sh_sb[mod][:, si - 3, :, :]
                    e_base = 6 + 2 * (si - 3)
                # batched transpose: ki 0-3 in one [128,128] op, ki=4 alone
                tpA = psum_misc.tile([128, 128], F32, tag="misc",
                                     name=nm("tpA"))
                nc.tensor.transpose(tpA[:], src_ap[:, 0:4, :], ident[:])
                featA = comb_pool.tile([128, 128], F32, tag="featb",
                                       name=nm("featA"))
                nc.scalar.copy(featA[:], tpA[:])
                tpB = psum_misc.tile([B, 128], F32, tag="misc", name=nm("tpB"))
                nc.tensor.transpose(tpB[:], src_ap[:, 4, :], ident[:])
                featB = comb_pool.tile([B, 128], F32, tag="featbB",
                                       name=nm("featB"))
                nc.scalar.copy(featB[:], tpB[:])
                for eloc in (0, 1):
                    e = e_base + eloc
                    nc.vector.scalar_tensor_tensor(
                        out=comb4[:], in0=featA[:, 64 * eloc:64 * eloc + 64],
                        scalar=gate4[:, e:e + 1], in1=comb4[:],
                        op0=ALU.mult, op1=ALU.add)
                    nc.vector.scalar_tensor_tensor(
                        out=combB[:], in0=featB[:, 64 * eloc:64 * eloc + 64],
                        scalar=gate[:, e:e + 1], in1=combB[:],
                        op0=ALU.mult, op1=ALU.add)
            # back to feature-major: one transpose each; ki selected by
            # free-dim column slices in the MLP matmuls
            tc4 = psum_misc.tile([64, 128], F32, tag="misc", name=nm("tc4"))
            nc.tensor.transpose(tc4[:], comb4[:], ident[:])
            combT = combt_pool.tile([64, 128], F32, tag="combT",
                                    name=nm("combT"))
            nc.scalar.copy(combT[:], tc4[:])
            tcB = psum_misc.tile([64, B], F32, tag="misc", name=nm("tcB"))
            nc.tensor.transpose(tcB[:], combB[:], ident[0:B, 0:B])
            combTB = combt_pool.tile([64, B], F32, tag="combTB",
                                     name=nm("combTB"))
            nc.scalar.copy(combTB[:], tcB[:])
            hhT = small.tile([128, 3, B], F32, tag="hhT")
            for mc in range(3):
                hh_ps = psum_misc.tile([128, B], F32, tag="misc",
                                       name=nm("hhps"))
                for ki in range(5):
                    rhs = (combT[:, 32 * ki:32 * ki + 32] if ki < 4
                           else combTB[:])
                    nc.tensor.matmul(
                        hh_ps[:], cw1[:, mod, ki, mc, :], rhs,
                        start=(ki == 0), stop=(ki == 4))
                nc.scalar.activation(
                    hhT[:, mc, :], hh_ps[:], ACTF.Relu,
                    bias=cb1[:, mod, mc:mc + 1])
            lg_ps = psum_misc.tile([1, B], F32, tag="misc")
            for kc in range(3):
                nc.tensor.matmul(
                    lg_ps[:], cw2[:, mod, kc:kc + 1], hhT[:, kc, :],
                    start=(kc == 0), stop=(kc == 2))
            lg = small.tile([1, B], F32, tag="lg")
            nc.scalar.activation(lg[:], lg_ps[:], ACTF.Identity,
                                 bias=cb2[0:1, mod, :])
            nc.sync.dma_start(out_dram[mod], lg[:])

        # ================= main program =================
        for rep in range(reps):
          # half-batch shared slots lead: DMA each slot's slice just before
          # its convs so the PE starts ~8us in, not behind the bulk loads
          xtsh = xt_pool.tile([128, NHS, DC, HW2], FP8, tag="xtsh")
          hs_done = True
          for i in range(NHS):
              for dcc in range(DC):
                  nc.sync.dma_start(
                      xtsh[:, i, dcc, :], di["xt_sh"][i, dcc])
              s = 6 + i
              if s in slots:
                  conv_slot(
                      s,
                      lambda q, r0, i=i: xtsh[:, i, 2 * q:2 * q + 2,
                                              r0:r0 + 2 * LP],
                      HBB)
              else:
                  hs_done = False
              if i == 0:
                  # heavy consts queue behind the first slot's data
                  deferred_consts()
          if hs_done:
              for i in range(NHS):
                  nc.sync.dma_start(ag_in[:, i, :, :],
                                    feat[6 + i][:, :, 0:HB])
              if not no_cc:
                  nc.gpsimd.collective_compute(
                      "AllGather", ALU.bypass,
                      replica_groups=[list(range(NCORES))],
                      ins=[ag_in[:].opt()],
                      outs=[ag_out[:].opt()])

          xts = {}
          for mod in (0, 1):
              xts[mod] = xt_pool.tile([128, DC, BLP2], FP8, tag="xt",
                                      name=nm("xt"))

          def load_xt(mod):
              for dcc in range(DC):
                  nc.sync.dma_start(
                      xts[mod][:, dcc, :], di[f"xt_{MODN[mod][0]}"][dcc])

          def conv_dom(s, mod):
              if s in slots:
                  conv_slot(
                      s,
                      lambda q, r0, mod=mod: xts[mod][:, 2 * q:2 * q + 2,
                                                      r0:r0 + 2 * LP],
                      nbb)

          def assemble_sh():
              if hs_done and not no_cc:
                  for g in range(2 * DC * 2):   # 24 half-tile units
                      csrc, i = g // NHS, g % NHS
                      t, h = g // 2, g % 2
                      mod, tm = (0, t) if t < 6 else (1, t - 6)
                      nc.sync.dma_start(
                          sh_sb[mod][:, tm, :, HB * h:HB * h + HB],
                          ag_out[csrc][:, i, :, :])
              else:
                  for mod in (0, 1):
                      nc.vector.memset(sh_sb[mod][:], 0.0)

          # mod 0: gating stages woven between the three domain conv slots
          load_xt(0)
          st0 = sc_stage1(0, xts[0])
          conv_dom(0, 0)
          sc_stage2(0, st0)
          conv_dom(1, 0)
          gate_sb[0] = sc_stage3(0, st0)
          conv_dom(2, 0)
          # mod-0 combine fills the PE while xts[1] reloads (WAR on xt buf)
          assemble_sh()
          combine_mlp(0, gate_sb[0])
          load_xt(1)
          st1 = sc_stage1(1, xts[1])
          conv_dom(3, 1)
          sc_stage2(1, st1)
          conv_dom(4, 1)
          gate_sb[1] = sc_stage3(1, st1)
          conv_dom(5, 1)
          combine_mlp(1, gate_sb[1])


# ---------------------------------------------------------------------------
# Host-side preparation
# ---------------------------------------------------------------------------

def f32(x):
    return np.ascontiguousarray(np.asarray(x, np.float32))


def q8(x, scale):
    return np.clip(x * np.float32(scale), -240.0, 240.0).astype(E4M3)


def host_prep(inputs):
    """Builds the 8 per-core input maps."""
    xs = {0: f32(inputs["text_feature"]), 1: f32(inputs["image_feature"])}
    cat = np.asarray(inputs["category"], np.int64)

    shared = {}
    xT, xN = {}, {}
    for m in (0, 1):
        xp = np.zeros((B, LP, D), np.float32)
        xp[:, :L, :] = xs[m]
        flat = xp.reshape(BLP, D)
        xt8 = np.zeros((DC, 128, BLP2), E4M3)
        xt8[:, :, :BLP] = q8(
            np.ascontiguousarray(flat.T.reshape(DC, 128, BLP)), SX)
        xT[m] = xt8
        xN[m] = q8(np.ascontiguousarray(flat.reshape(TC, 128, D)), SX)
    shared["xt_t"], shared["xt_i"] = xT[0], xT[1]
    shared["xn_t"], shared["xn_i"] = xN[0], xN[1]

    r = np.arange(BLP)
    bi = np.zeros((BLP, B), np.float32)
    valid = (r % LP) < L
    bi[valid, (r[valid] // LP)] = 1.0
    shared["b_ind"] = np.ascontiguousarray(
        bi.reshape(TC, 128, B).transpose(1, 0, 2)).astype(BF16)
    shared["rep4"] = np.ascontiguousarray(
        np.tile(np.eye(B, dtype=np.float32), (1, 4)))

    mask2 = np.zeros((2, B, LP), np.float32)
    mask2[0, :, :L] = (f32(inputs["masks"]) > 0).astype(np.float32)
    mask2[1, :, :L] = 1.0
    shared["mask2"] = mask2

    # aw[p, q, i, m]: contract index d = (2q+i)*128+p, m = modality (pad 16)
    awp = np.zeros((128, QC, 2, 16), np.float32)
    for m in (0, 1):
        awp[:, :, :, m] = f32(inputs[f"{MODN[m]}_aw"]).reshape(
            QC, 2, 128).transpose(2, 0, 1)
    shared["aw"] = q8(awp, SW)

    dom_b = f32(inputs["domain_emb"])[cat]
    shared["dom_embT"] = np.ascontiguousarray(
        dom_b.T.reshape(DC, 128, B).transpose(1, 0, 2)).astype(BF16)
    shared["ident128"] = np.eye(128, dtype=np.float32)

    in_maps = []
    for c in range(NCORES):
        d = dict(shared)
        # per-core half-batch shared-slot input: columns of the assigned
        # (modality, batch-half) slice of xt
        xsh = np.zeros((NHS, DC, 128, HW2), E4M3)
        for i in range(NHS):
            mod, _, h = slot_experts(c, 6 + i)
            xsh[i] = xT[mod][:, :, HB * LP * h:HB * LP * h + HW2]
        d["xt_sh"] = xsh
        for k in KS:
            wk = np.zeros((T, 128, k, QC, 2, 128), np.float32)
            for s in range(T):
                mod, es, _ = slot_experts(c, s)
                wsrc = f32(inputs[f"{MODN[mod]}_cw_k{k}"])
                for el, e in enumerate(es):
                    w_e = wsrc[e]       # [FK, D, k]
                    # wk[p, j, q, i, ef]: contract index d = (2q+i)*128+p
                    wt = w_e.transpose(1, 2, 0).reshape(
                        QC, 2, 128, k, FK).transpose(2, 3, 0, 1, 4)
                    wk[s, :, :, :, :, el * 64:(el + 1) * 64] = wt
            d[f"w_k{k}"] = q8(wk, SW)
        cb = np.zeros((128, T, len(KS)), np.float32)
        for s in range(T):
            mod, es, _ = slot_experts(c, s)
            cbs = f32(inputs[f"{MODN[mod]}_cb"])
            for el, e in enumerate(es):
                cb[el * 64:(el + 1) * 64, s, :] = cbs[:, e, :].T
        d["cbias"] = cb * np.float32(SXW)
        gw1 = np.stack([f32(inputs[f"{MODN[m]}_gw1"])[c] for m in (0, 1)]
                       ).reshape(2, 12, 128, DC, 128)
        gw1[:, 0:6] *= 1.0 / SX   # undo the fp8 xn scale on the pooled rows
        d["gw1"] = gw1.astype(BF16)
        gb1 = np.stack([f32(inputs[f"{MODN[m]}_gb1"])[c] for m in (0, 1)])
        d["gb1"] = np.ascontiguousarray(np.repeat(gb1[:, None, :], B, axis=1))
        d["gw2"] = np.ascontiguousarray(
            np.stack([f32(inputs[f"{MODN[m]}_gw2"])[c] for m in (0, 1)]
                     ).reshape(2, DC, 128, GATE_E).transpose(0, 2, 1, 3))
        gb2 = np.stack([f32(inputs[f"{MODN[m]}_gb2"])[c] for m in (0, 1)])
        d["gb2"] = np.ascontiguousarray(np.repeat(gb2[:, None, :], B, axis=1))
        cw1 = np.stack([f32(inputs[f"{MODN[m]}_cw1"])[c] for m in (0, 1)])
        d["cw1"] = np.ascontiguousarray(
            (cw1 / np.float32(SXW)).reshape(2, 5, 64, 3, 128)
            .transpose(0, 2, 1, 3, 4))
        d["cb1"] = np.ascontiguousarray(
            np.stack([f32(inputs[f"{MODN[m]}_cb1"])[c] for m in (0, 1)]
                     ).reshape(2, 3, 128).transpose(0, 2, 1))
        cw2 = np.stack([f32(inputs[f"{MODN[m]}_cw2"])[c] for m in (0, 1)])
        d["cw2"] = np.ascontiguousarray(
            cw2.reshape(2, 3, 128).transpose(0, 2, 1))
        d["cb2"] = np.stack([f32(inputs[f"{MODN[m]}_cb2"])[c] for m in (0, 1)]
                            ).reshape(2, 1, 1).copy()
        in_maps.append(d)
    return in_maps, cat


_NC_CACHE = {}


def _get_nc():
    if "nc" not in _NC_CACHE:
        _NC_CACHE["nc"] = build_nc()
    return _NC_CACHE["nc"]


def kernel(**inputs):
    nc = _get_nc()
    in_maps, cat = host_prep(inputs)
    res = bass_utils.run_bass_kernel_spmd(
        nc, in_maps, core_ids=list(range(NCORES)))
    logits_all = np.stack([res.results[c]["logits"][:, 0, :]
                           for c in range(NCORES)])      # [8, 2, 32]
    sel = logits_all[cat, :, np.arange(B)]               # [32, 2]
    out = 1.0 / (1.0 + np.exp(-sel.astype(np.float64)))
    t_pred = out[:, 0].astype(np.float32)
    i_pred = out[:, 1].astype(np.float32)
    return t_pred, i_pred


if __name__ == "__main__":
    import time
    t0 = time.time()
    build_nc()
    print(f"build+compile: {time.time()-t0:.1f}s")

